# revision 56
# baseline (speedup 1.0000x reference)
"""Trainium2 Bass kernel for BertSelfAttention with relative position embeddings.

Math (per batch b=1, S=384, H=768, NH=12, D=64):
  q/k/v = hs @ W{q,k,v}.T          (biases are zero in this problem -> skipped)
  a_c[h,q,k] = sum_d (q+u)[h,q,d] * k[h,k,d]
  b_d[h,q,k] = sum_F rel[q,k,F] * g[q,h,F],  g[q,h,F] = sum_d (q+v)[h,q,d]*Wr[h*64+d,F]
  out = softmax((a_c+b_d)/8 + mask) @ v

The g-reassociation avoids projecting the giant rel tensor through Wr
(64x FLOP reduction); the kernel is then memory-bound on streaming rel.

Key design points (all reshapes/casts host-side, all FLOPs on device):
  * rel is pre-transposed on host to [F, k] layout per q row and quantized
    to f8e3 (e3m4: fits N(0,1) data, 4 mantissa bits) -- halves the
    dominant DMA stream vs bf16.  Wq/Wk/Wr are x64-scaled into f8e3 range
    too (scores come out x4096; the exp scale folds it back).  Wv and hs
    stay bf16 -- their quantization showed up 1:1 in the output error.
  * b_d uses the rel tile as the STATIONARY matmul operand and g as the
    moving one: out [k(128), h(12)] per (q, ktile, Fchunk).  No PE
    transposes anywhere; a_c is precomputed (mask folded in at eviction)
    and added into the same PSUM accumulation via an identity matmul.
  * Every matmul is K=128 at partition offset 0 (consecutive K=64 matmuls
    with alternating partition offsets wedge the PE); per-head d-
    contractions use block-diagonal qu/qv operands (head pair per chunk).
  * ctx uses exp as the stationary operand -> [q, d] directly; softmax
    normalization is two wide DVE tensor_tensor muls with a stride-0
    broadcast of 1/den (per-h scalars), not 12 per-head ops.
  * All DMAs are HWDGE on the idle SP queue; setup tensors are packed into
    4 DMAs; rel streams in 16 blocks of 3 q rows with an 8-deep prefetch
    pool; den/ctx/out run in 4 phases so the softmax tail overlaps the
    stream and only one 3-row block remains after the last rel transfer.

Sharding: query axis across 8 cores (48 q rows each), no collectives.
Timeline cost model: 58.3us/core vs 225.9us baseline (3.9x); the DMA
device is busy 49.6us of that (the fp8 rel stream is 39.3us of it).
"""

import numpy as np

S, H, NH, D = 384, 768, 12, 64
NCORES = 8
SQ = S // NCORES          # 48 q rows per core
KT = S // 128             # 3 k tiles
FC = H // 128             # 6 feature chunks
P = 128
QB = 2                    # q rows per rel DMA block
NQB = SQ // QB            # 12 blocks

REL_FP8 = True            # rel stream dtype: f8e3 (e3m4) vs bf16
W_FP8 = True              # Wq/Wr in x64-scaled f8e3 (Wk x64 in bf16 to match scale)
SCL = 4096.0 if W_FP8 else 1.0   # score scale: (64*Wq)(64*Wk) and (64*qv)(64*Wr)

_CACHED = {}


def build_kernel():
    import concourse.bacc as bacc
    import concourse.tile as tile
    from concourse import mybir
    from concourse.masks import make_identity

    f32 = mybir.dt.float32
    bf16 = mybir.dt.bfloat16
    f8 = mybir.dt.float8e3 if REL_FP8 else mybir.dt.bfloat16
    f8w = mybir.dt.float8e3 if W_FP8 else mybir.dt.bfloat16
    EXP = mybir.ActivationFunctionType.Exp
    COPY = mybir.ActivationFunctionType.Copy

    nc = bacc.Bacc("TRN2", target_bir_lowering=False)

    # host-prearranged layouts (see make_in_maps); setup tensors are packed
    # into few DMAs (each DMA costs ~650ns on the serialized HWDGE pipe)
    relT = nc.dram_tensor("relT", [NQB, P, QB, FC, S], f8, kind="ExternalInput")
    # qkr = [wqT | wkT | wr] packed on the second axis
    qkr = nc.dram_tensor("qkr", [P, 3, FC, H], f8w, kind="ExternalInput")
    # hsc = [hsT | hslT] packed on the last axis
    hsc = nc.dram_tensor("hsc", [P, FC, S + SQ], bf16, kind="ExternalInput")
    wvT = nc.dram_tensor("wvT", [P, FC, H], bf16, kind="ExternalInput")
    # uvm = [u | v | 8*SCL*mask] packed on the last axis
    uvm = nc.dram_tensor("uvm", [P, 2 * FC + KT], f32, kind="ExternalInput")
    out = nc.dram_tensor("out", [SQ, H], bf16, kind="ExternalOutput")

    with tile.TileContext(nc) as tc:
        with (
            tc.tile_pool(name="persist", bufs=1) as persist,
            tc.tile_pool(name="relbf", bufs=8) as relbf,
        ):
            # ---- setup DMAs first, then the rel stream ----
            qkr_sb = persist.tile([P, 3, FC, H], f8w)
            nc.sync.dma_start(out=qkr_sb, in_=qkr[:, :, :, :])
            wq_sb, wk_sb, wr_sb = qkr_sb[:, 0], qkr_sb[:, 1], qkr_sb[:, 2]
            hsc_sb = persist.tile([P, FC, S + SQ], bf16)
            nc.sync.dma_start(out=hsc_sb, in_=hsc[:, :, :])
            hsT_sb, hslT_sb = hsc_sb[:, :, :S], hsc_sb[:, :, S:]
            uvm_sb = persist.tile([P, 2 * FC + KT], f32)
            nc.sync.dma_start(out=uvm_sb, in_=uvm[:, :])
            u_sb, v_sb = uvm_sb[:, :FC], uvm_sb[:, FC:2 * FC]
            mask_sb = uvm_sb[:, 2 * FC:]
            wv_sb = persist.tile([P, FC, H], bf16)
            nc.sync.dma_start(out=wv_sb, in_=wvT[:, :, :])

            rel_tiles = []
            for qb in range(NQB):
                rbf = relbf.tile([P, QB, FC, S], f8, tag="rbf")
                eng = nc.sync if qb % 2 == 0 else nc.scalar
                eng.dma_start(out=rbf, in_=relT[qb])
                rel_tiles.append(rbf)

            ident_bf = persist.tile([P, P], bf16)
            make_identity(nc, ident_bf)
            ones_bf = persist.tile([P, 1], bf16)
            nc.vector.memset(ones_bf, 1.0)

            # ---- projections ----
            # qu/qv are built BLOCK-DIAGONAL per oc-chunk (each 128-row chunk
            # covers head pair (2*oc, 2*oc+1)): rows 0:64 feed free cols 0:48
            # (head 2*oc), rows 64:128 feed cols 48:96 (head 2*oc+1). This
            # keeps every matmul K=128 at partition offset 0 (K=64 matmuls
            # with alternating partition offsets wedge the PE).
            quBD = persist.tile([P, FC, 2, SQ], bf16)
            qvBD = persist.tile([P, FC, 2, SQ], bf16)
            nc.vector.memset(quBD, 0.0)
            nc.vector.memset(qvBD, 0.0)
            with tc.tile_pool(name="pproj", bufs=4, space="PSUM") as pproj:
                for oc in range(FC):
                    pq = pproj.tile([P, SQ], f32, tag="pp")
                    for ic in range(FC):
                        nc.tensor.matmul(
                            pq, wq_sb[:, ic, oc * P:(oc + 1) * P], hslT_sb[:, ic, :],
                            start=(ic == 0), stop=(ic == FC - 1))
                    for par in range(2):
                        pr = slice(par * 64, (par + 1) * 64)
                        nc.vector.tensor_scalar_add(
                            out=quBD[pr, oc, par, :], in0=pq[pr, :],
                            scalar1=u_sb[pr, oc:oc + 1])
                        nc.vector.tensor_scalar_add(
                            out=qvBD[pr, oc, par, :], in0=pq[pr, :],
                            scalar1=v_sb[pr, oc:oc + 1])

                # gT[F, h, q] = sum_d Wr[h*64+d, F] * qv[h*64+d, q]
                # one K=128 matmul per (ft, oc) covers head pair (2oc, 2oc+1)
                gT = persist.tile([P, FC, NH, SQ], bf16)
                for ft in range(FC):
                    for hg in range(2):
                        pg = pproj.tile([P, 6 * SQ], f32, tag="pp")
                        for i in range(3):
                            oc = hg * 3 + i
                            nc.tensor.matmul(
                                pg[:, i * 2 * SQ:(i + 1) * 2 * SQ],
                                wr_sb[:, oc, ft * P:(ft + 1) * P],
                                qvBD[:, oc, :, :].rearrange("p t q -> p (t q)"),
                                start=True, stop=True)
                        if hg == 0:
                            nc.vector.tensor_copy(
                                out=gT[:, ft, hg * 6:(hg + 1) * 6, :].rearrange(
                                    "p h q -> p (h q)"), in_=pg)
                        else:
                            nc.scalar.activation(
                                out=gT[:, ft, hg * 6:(hg + 1) * 6, :].rearrange(
                                    "p h q -> p (h q)"), in_=pg, func=COPY)

                # kT[o, k] (o on partitions within oc)
                kT_sb = persist.tile([P, FC, S], bf16)
                for oc in range(FC):
                    pk = pproj.tile([P, S], f32, tag="pp")
                    for ic in range(FC):
                        nc.tensor.matmul(
                            pk, wk_sb[:, ic, oc * P:(oc + 1) * P], hsT_sb[:, ic, :],
                            start=(ic == 0), stop=(ic == FC - 1))
                    nc.scalar.activation(out=kT_sb[:, oc, :], in_=pk, func=COPY)

                # a_cT[k, kt, h, q] = sum_d k[h*64+d, k] * qu[h*64+d, q], + 8*mask
                # (mask is pre-scaled by 8 on host; exp applies the 1/8)
                # same K=128 block-diagonal trick as gT
                a_cT = persist.tile([P, KT, NH, SQ], bf16)
                for kt in range(KT):
                    for hg in range(2):
                        pac = pproj.tile([P, 6 * SQ], f32, tag="pp")
                        for i in range(3):
                            oc = hg * 3 + i
                            nc.tensor.matmul(
                                pac[:, i * 2 * SQ:(i + 1) * 2 * SQ],
                                kT_sb[:, oc, kt * P:(kt + 1) * P],
                                quBD[:, oc, :, :].rearrange("p t q -> p (t q)"),
                                start=True, stop=True)
                        nc.vector.tensor_scalar_add(
                            out=a_cT[:, kt, hg * 6:(hg + 1) * 6, :].rearrange(
                                "p h q -> p (h q)"),
                            in0=pac, scalar1=mask_sb[:, kt:kt + 1])

            # ---- rel stream: per 4-row q block; den/ctx/out emitted per
            # half (q 0:24 after block 5, q 24:48 after block 11) so the
            # softmax tail overlaps the second half of the stream. The val
            # projection (first needed by half-0 ctx) is emitted after the
            # first half's stream blocks: it fills PE idle gaps between
            # DMA-paced blocks instead of delaying the stream start. ----
            # phases: (block range, q range); the last phase is a single
            # block so the post-stream tail is minimal
            PHASES = [(0, 8), (8, 12), (12, 15), (15, 16)]
            HD = 8 * D      # out split matches the two ctx psum banks
            val_sb = persist.tile([P, KT, H], bf16)
            expT = persist.tile([P, SQ, KT, NH], bf16)
            with (
                tc.tile_pool(name="psc", bufs=2, space="PSUM") as pscp,
                tc.tile_pool(name="pden", bufs=2, space="PSUM") as pden,
                tc.tile_pool(name="pctx", bufs=2, space="PSUM") as pctx,
            ):
                for ph, (b0, b1) in enumerate(PHASES):
                    for qb in range(b0, b1):
                        rbf = rel_tiles[qb]
                        psc = pscp.tile([P, QB * KT * NH], f32, tag="sc")
                        for j in range(QB):
                            q = qb * QB + j
                            for kt in range(KT):
                                off = (j * KT + kt) * NH
                                for fc in range(FC):
                                    nc.tensor.matmul(
                                        psc[:, off:off + NH],
                                        rbf[:, j, fc, kt * P:(kt + 1) * P],
                                        gT[:, fc, :, q],
                                        start=(fc == 0), stop=False)
                                nc.tensor.matmul(
                                    psc[:, off:off + NH], ident_bf,
                                    a_cT[:, kt, :, q], start=False, stop=True)
                        # exp((a_c + b_d + 8*mask)/8)
                        nc.scalar.activation(
                            out=expT[:, qb * QB:(qb + 1) * QB, :, :].rearrange(
                                "p j kt h -> p (j kt h)"),
                            in_=psc, func=EXP, scale=1.0 / (np.sqrt(D).item() * SCL))

                    if ph == 0:
                        # val[k, o] (k on partitions within kt): emitted after
                        # the first phase's blocks so it fills PE idle gaps
                        # between DMA-paced blocks, not the stream start
                        for kt in range(KT):
                            for vh in range(2):
                                pv = pscp.tile([P, H // 2], f32, tag="sc")
                                for ic in range(FC):
                                    nc.tensor.matmul(
                                        pv, hsT_sb[:, ic, kt * P:(kt + 1) * P],
                                        wv_sb[:, ic, vh * (H // 2):(vh + 1) * (H // 2)],
                                        start=(ic == 0), stop=(ic == FC - 1))
                                nc.vector.tensor_copy(
                                    out=val_sb[:, kt, vh * (H // 2):(vh + 1) * (H // 2)],
                                    in_=pv)

                    # den + ctx matmuls run unhindered into packed psum banks;
                    # the normalization muls use engine-disjoint out tiles
                    # (alternating engines on one tile serializes via WAW sems)
                    q0, nq = b0 * QB, (b1 - b0) * QB
                    qs = slice(q0, q0 + nq)
                    out_sb = persist.tile([nq, H], bf16, name=f"out_sb{ph}")
                    den_r = persist.tile([nq, NH], f32, name=f"denr{ph}")
                    pd = pden.tile([nq, NH], f32, tag="den")
                    for h in range(NH):
                        for kt in range(KT):
                            nc.tensor.matmul(
                                pd[:, h:h + 1], expT[:, qs, kt, h], ones_bf,
                                start=(kt == 0), stop=(kt == KT - 1))
                    nc.vector.reciprocal(out=den_r, in_=pd)

                    pc0 = pctx.tile([nq, 8 * D], f32, tag="cb0", name="pc0")
                    pc1 = pctx.tile([nq, 4 * D], f32, tag="cb1", name="pc1")
                    pcs = [pc0, pc1]
                    # small bank (pc1) first: its mul overlaps the pc0 matmuls
                    for h in list(range(8, NH)) + list(range(8)):
                        bank, off = (pcs[0], h * D) if h < 8 else (pcs[1], (h - 8) * D)
                        for kt in range(KT):
                            nc.tensor.matmul(
                                bank[:, off:off + D], expT[:, qs, kt, h],
                                val_sb[:, kt, h * D:(h + 1) * D],
                                start=(kt == 0), stop=(kt == KT - 1))
                    # normalize with two wide tensor_tensor muls; den_r is
                    # free-dim-broadcast (stride 0 over d) to [nq, h, 64];
                    # both on DVE (same engine: no cross-engine WAW on out_sb)
                    den_b = den_r.rearrange("q (h o) -> q h o", o=1)
                    nc.vector.tensor_mul(
                        out=out_sb[:, HD:].rearrange("q (h o) -> q h o", o=D),
                        in0=pc1, in1=den_b[:, 8:12, :].broadcast_to([nq, 4, D]))
                    nc.vector.tensor_mul(
                        out=out_sb[:, :HD].rearrange("q (h o) -> q h o", o=D),
                        in0=pc0, in1=den_b[:, 0:8, :].broadcast_to([nq, 8, D]))

                    nc.sync.dma_start(out=out[qs, :], in_=out_sb)

    nc.compile()
    return nc


def make_in_maps(inputs):
    import ml_dtypes
    bf = ml_dtypes.bfloat16
    f8 = ml_dtypes.float8_e3m4 if REL_FP8 else ml_dtypes.bfloat16

    hs = np.asarray(inputs["hidden_states"], np.float32)[0]          # [S, H]
    rel = np.asarray(inputs["rel_embedding"], np.float32)[0]         # [S, S, H]
    msk = np.asarray(inputs["attention_mask"], np.float32).reshape(S)

    # rel -> per-core [NQB, P, QB, FC, S] f8, relT[qb,p,j,fc,k] = rel[q, k, fc*128+p]
    rel_q = rel.astype(f8).reshape(NCORES, NQB, QB, S, FC, P)
    rel_t = np.ascontiguousarray(rel_q.transpose(0, 1, 5, 2, 4, 3))

    f8w = ml_dtypes.float8_e3m4 if W_FP8 else bf
    wscl = 64.0 if W_FP8 else 1.0

    def t_po(a, dt=bf):  # [O, I] -> [P, FC(I), O] with partition = i within chunk
        return np.ascontiguousarray(
            a.astype(dt).T.reshape(FC, P, -1).transpose(1, 0, 2))

    def t_nat(a, dt=bf):  # [O, I] -> [P, FC(O), I] natural rows on partitions
        return np.ascontiguousarray(
            a.astype(dt).reshape(FC, P, -1).transpose(1, 0, 2))

    qkr = np.ascontiguousarray(np.stack([
        t_po(np.asarray(inputs["Wq"], np.float32) * wscl, f8w),
        t_po(np.asarray(inputs["Wk"], np.float32) * wscl, f8w),
        t_nat(np.asarray(inputs["Wr"], np.float32) * wscl, f8w),
    ], axis=1))                                                       # [P,3,FC,H]
    uvm = np.ascontiguousarray(np.concatenate([
        np.asarray(inputs["u"], np.float32).reshape(FC, P).T * wscl,
        np.asarray(inputs["v"], np.float32).reshape(FC, P).T * wscl,
        (msk * 8.0 * SCL).reshape(KT, P).T,
    ], axis=1))                                                       # [P,2FC+KT]
    hsT = t_po(hs)                                                    # [P, FC, S]
    common = {"qkr": qkr, "uvm": uvm,
              "wvT": t_po(np.asarray(inputs["Wv"], np.float32))}
    in_maps = []
    for c in range(NCORES):
        sl = slice(c * SQ, (c + 1) * SQ)
        in_maps.append({
            **common,
            "hsc": np.ascontiguousarray(
                np.concatenate([hsT, t_po(hs[sl])], axis=2)),
            "relT": rel_t[c],
        })
    return in_maps


def kernel(**inputs):
    if "nc" not in _CACHED:
        _CACHED["nc"] = build_kernel()
    nc = _CACHED["nc"]
    in_maps = make_in_maps(inputs)

    from concourse.bass_utils import run_bass_kernel_spmd
    res = run_bass_kernel_spmd(nc, in_maps, list(range(NCORES)))
    parts = [np.asarray(res.results[c]["out"]).astype(np.float32)
             for c in range(NCORES)]
    return np.concatenate(parts, axis=0)[None]


# revision 57
# speedup vs baseline: 1.0379x; 1.0379x over previous
"""Trainium2 Bass kernel for BertSelfAttention with relative position embeddings.

Math (per batch b=1, S=384, H=768, NH=12, D=64):
  q/k/v = hs @ W{q,k,v}.T          (biases are zero in this problem -> skipped)
  a_c[h,q,k] = sum_d (q+u)[h,q,d] * k[h,k,d]
  b_d[h,q,k] = sum_F rel[q,k,F] * g[q,h,F],  g[q,h,F] = sum_d (q+v)[h,q,d]*Wr[h*64+d,F]
  out = softmax((a_c+b_d)/8 + mask) @ v

The g-reassociation avoids projecting the giant rel tensor through Wr
(64x FLOP reduction); the kernel is then memory-bound on streaming rel.

Key design points (all reshapes/casts host-side, all FLOPs on device):
  * rel is pre-transposed on host to [F, k] layout per q row and quantized
    to f8e3 (e3m4: fits N(0,1) data, 4 mantissa bits) -- halves the
    dominant DMA stream vs bf16.  Wq/Wk/Wr are x64-scaled into f8e3 range
    too (scores come out x4096; the exp scale folds it back).  Wv and hs
    stay bf16 -- their quantization showed up 1:1 in the output error.
  * b_d uses the rel tile as the STATIONARY matmul operand and g as the
    moving one: out [k(128), h(12)] per (q, ktile, Fchunk).  No PE
    transposes anywhere; a_c is precomputed (mask folded in at eviction)
    and added into the same PSUM accumulation via an identity matmul.
  * Every matmul is K=128 at partition offset 0 (consecutive K=64 matmuls
    with alternating partition offsets wedge the PE); per-head d-
    contractions use block-diagonal qu/qv operands (head pair per chunk).
  * ctx uses exp as the stationary operand -> [q, d] directly; softmax
    normalization is two wide DVE tensor_tensor muls with a stride-0
    broadcast of 1/den (per-h scalars), not 12 per-head ops.
  * All DMAs are HWDGE on the idle SP queue; setup tensors are packed into
    4 DMAs; rel streams in 16 blocks of 3 q rows with an 8-deep prefetch
    pool; den/ctx/out run in 4 phases so the softmax tail overlaps the
    stream and only one 3-row block remains after the last rel transfer.

Sharding: query axis across 8 cores (48 q rows each), no collectives.
Timeline cost model: 58.3us/core vs 225.9us baseline (3.9x); the DMA
device is busy 49.6us of that (the fp8 rel stream is 39.3us of it).
"""

import numpy as np

S, H, NH, D = 384, 768, 12, 64
NCORES = 8
SQ = S // NCORES          # 48 q rows per core
KT = S // 128             # 3 k tiles
FC = H // 128             # 6 feature chunks
P = 128
QB = 2                    # q rows per rel DMA block
NQB = SQ // QB            # 12 blocks

REL_FP8 = True            # rel stream dtype: f8e3 (e3m4) vs bf16
W_FP8 = True              # Wq/Wr in x64-scaled f8e3 (Wk x64 in bf16 to match scale)
SCL = 4096.0 if W_FP8 else 1.0   # score scale: (64*Wq)(64*Wk) and (64*qv)(64*Wr)

_CACHED = {}


def build_kernel():
    import concourse.bacc as bacc
    import concourse.tile as tile
    from concourse import mybir
    from concourse.masks import make_identity

    f32 = mybir.dt.float32
    bf16 = mybir.dt.bfloat16
    f8 = mybir.dt.float8e3 if REL_FP8 else mybir.dt.bfloat16
    f8w = mybir.dt.float8e3 if W_FP8 else mybir.dt.bfloat16
    EXP = mybir.ActivationFunctionType.Exp
    COPY = mybir.ActivationFunctionType.Copy

    nc = bacc.Bacc("TRN2", target_bir_lowering=False)

    # host-prearranged layouts (see make_in_maps); setup tensors are packed
    # into few DMAs (each DMA costs ~650ns on the serialized HWDGE pipe)
    relT = nc.dram_tensor("relT", [NQB, P, QB, FC, S], f8, kind="ExternalInput")
    # qkr = [wqT | wkT | wr] packed on the second axis
    qkr = nc.dram_tensor("qkr", [P, 3, FC, H], f8w, kind="ExternalInput")
    # hsc = [hsT | hslT] packed on the last axis
    hsc = nc.dram_tensor("hsc", [P, FC, S + SQ], bf16, kind="ExternalInput")
    wvT = nc.dram_tensor("wvT", [P, FC, H], bf16, kind="ExternalInput")
    # uvm = [u | v | 8*SCL*mask] packed on the last axis
    uvm = nc.dram_tensor("uvm", [P, 2 * FC + KT], f32, kind="ExternalInput")
    out = nc.dram_tensor("out", [SQ, H], bf16, kind="ExternalOutput")

    with tile.TileContext(nc) as tc:
        with (
            tc.tile_pool(name="persist", bufs=1) as persist,
            tc.tile_pool(name="relbf", bufs=8) as relbf,
        ):
            # ---- setup DMAs first, then the rel stream ----
            qkr_sb = persist.tile([P, 3, FC, H], f8w)
            nc.sync.dma_start(out=qkr_sb, in_=qkr[:, :, :, :])
            wq_sb, wk_sb, wr_sb = qkr_sb[:, 0], qkr_sb[:, 1], qkr_sb[:, 2]
            hsc_sb = persist.tile([P, FC, S + SQ], bf16)
            nc.sync.dma_start(out=hsc_sb, in_=hsc[:, :, :])
            hsT_sb, hslT_sb = hsc_sb[:, :, :S], hsc_sb[:, :, S:]
            uvm_sb = persist.tile([P, 2 * FC + KT], f32)
            nc.sync.dma_start(out=uvm_sb, in_=uvm[:, :])
            u_sb, v_sb = uvm_sb[:, :FC], uvm_sb[:, FC:2 * FC]
            mask_sb = uvm_sb[:, 2 * FC:]
            wv_sb = persist.tile([P, FC, H], bf16)
            nc.sync.dma_start(out=wv_sb, in_=wvT[:, :, :])

            rel_tiles = []
            for qb in range(NQB):
                rbf = relbf.tile([P, QB, FC, S], f8, tag="rbf")
                nc.sync.dma_start(out=rbf, in_=relT[qb])
                rel_tiles.append(rbf)

            ident_bf = persist.tile([P, P], bf16)
            make_identity(nc, ident_bf)
            ones_bf = persist.tile([P, 1], bf16)
            nc.vector.memset(ones_bf, 1.0)

            # ---- projections ----
            # qu/qv are built BLOCK-DIAGONAL per oc-chunk (each 128-row chunk
            # covers head pair (2*oc, 2*oc+1)): rows 0:64 feed free cols 0:48
            # (head 2*oc), rows 64:128 feed cols 48:96 (head 2*oc+1). This
            # keeps every matmul K=128 at partition offset 0 (K=64 matmuls
            # with alternating partition offsets wedge the PE).
            quBD = persist.tile([P, FC, 2, SQ], bf16)
            qvBD = persist.tile([P, FC, 2, SQ], bf16)
            nc.vector.memset(quBD, 0.0)
            nc.vector.memset(qvBD, 0.0)
            with tc.tile_pool(name="pproj", bufs=4, space="PSUM") as pproj:
                for oc in range(FC):
                    pq = pproj.tile([P, SQ], f32, tag="pp")
                    for ic in range(FC):
                        nc.tensor.matmul(
                            pq, wq_sb[:, ic, oc * P:(oc + 1) * P], hslT_sb[:, ic, :],
                            start=(ic == 0), stop=(ic == FC - 1))
                    for par in range(2):
                        pr = slice(par * 64, (par + 1) * 64)
                        nc.vector.tensor_scalar_add(
                            out=quBD[pr, oc, par, :], in0=pq[pr, :],
                            scalar1=u_sb[pr, oc:oc + 1])
                        nc.vector.tensor_scalar_add(
                            out=qvBD[pr, oc, par, :], in0=pq[pr, :],
                            scalar1=v_sb[pr, oc:oc + 1])

                # gT[F, h, q] = sum_d Wr[h*64+d, F] * qv[h*64+d, q]
                # one K=128 matmul per (ft, oc) covers head pair (2oc, 2oc+1)
                gT = persist.tile([P, FC, NH, SQ], bf16)
                for ft in range(FC):
                    for hg in range(2):
                        pg = pproj.tile([P, 6 * SQ], f32, tag="pp")
                        for i in range(3):
                            oc = hg * 3 + i
                            nc.tensor.matmul(
                                pg[:, i * 2 * SQ:(i + 1) * 2 * SQ],
                                wr_sb[:, oc, ft * P:(ft + 1) * P],
                                qvBD[:, oc, :, :].rearrange("p t q -> p (t q)"),
                                start=True, stop=True)
                        if hg == 0:
                            nc.vector.tensor_copy(
                                out=gT[:, ft, hg * 6:(hg + 1) * 6, :].rearrange(
                                    "p h q -> p (h q)"), in_=pg)
                        else:
                            nc.scalar.activation(
                                out=gT[:, ft, hg * 6:(hg + 1) * 6, :].rearrange(
                                    "p h q -> p (h q)"), in_=pg, func=COPY)

                # kT[o, k] (o on partitions within oc)
                kT_sb = persist.tile([P, FC, S], bf16)
                for oc in range(FC):
                    pk = pproj.tile([P, S], f32, tag="pp")
                    for ic in range(FC):
                        nc.tensor.matmul(
                            pk, wk_sb[:, ic, oc * P:(oc + 1) * P], hsT_sb[:, ic, :],
                            start=(ic == 0), stop=(ic == FC - 1))
                    nc.scalar.activation(out=kT_sb[:, oc, :], in_=pk, func=COPY)

                # a_cT[k, kt, h, q] = sum_d k[h*64+d, k] * qu[h*64+d, q], + 8*mask
                # (mask is pre-scaled by 8 on host; exp applies the 1/8)
                # same K=128 block-diagonal trick as gT
                a_cT = persist.tile([P, KT, NH, SQ], bf16)
                for kt in range(KT):
                    for hg in range(2):
                        pac = pproj.tile([P, 6 * SQ], f32, tag="pp")
                        for i in range(3):
                            oc = hg * 3 + i
                            nc.tensor.matmul(
                                pac[:, i * 2 * SQ:(i + 1) * 2 * SQ],
                                kT_sb[:, oc, kt * P:(kt + 1) * P],
                                quBD[:, oc, :, :].rearrange("p t q -> p (t q)"),
                                start=True, stop=True)
                        nc.vector.tensor_scalar_add(
                            out=a_cT[:, kt, hg * 6:(hg + 1) * 6, :].rearrange(
                                "p h q -> p (h q)"),
                            in0=pac, scalar1=mask_sb[:, kt:kt + 1])

            # ---- rel stream: per 4-row q block; den/ctx/out emitted per
            # half (q 0:24 after block 5, q 24:48 after block 11) so the
            # softmax tail overlaps the second half of the stream. The val
            # projection (first needed by half-0 ctx) is emitted after the
            # first half's stream blocks: it fills PE idle gaps between
            # DMA-paced blocks instead of delaying the stream start. ----
            # phases: (block range, q range); the last phase is a single
            # block so the post-stream tail is minimal
            PHASES = [(0, 8), (8, 12), (12, 15), (15, 16)]
            HD = 8 * D      # out split matches the two ctx psum banks
            val_sb = persist.tile([P, KT, H], bf16)
            expT = persist.tile([P, SQ, KT, NH], bf16)
            with (
                tc.tile_pool(name="psc", bufs=2, space="PSUM") as pscp,
                tc.tile_pool(name="pden", bufs=2, space="PSUM") as pden,
                tc.tile_pool(name="pctx", bufs=2, space="PSUM") as pctx,
            ):
                for ph, (b0, b1) in enumerate(PHASES):
                    for qb in range(b0, b1):
                        rbf = rel_tiles[qb]
                        psc = pscp.tile([P, QB * KT * NH], f32, tag="sc")
                        for j in range(QB):
                            q = qb * QB + j
                            for kt in range(KT):
                                off = (j * KT + kt) * NH
                                for fc in range(FC):
                                    nc.tensor.matmul(
                                        psc[:, off:off + NH],
                                        rbf[:, j, fc, kt * P:(kt + 1) * P],
                                        gT[:, fc, :, q],
                                        start=(fc == 0), stop=False)
                                nc.tensor.matmul(
                                    psc[:, off:off + NH], ident_bf,
                                    a_cT[:, kt, :, q], start=False, stop=True)
                        # exp((a_c + b_d + 8*mask)/8)
                        nc.scalar.activation(
                            out=expT[:, qb * QB:(qb + 1) * QB, :, :].rearrange(
                                "p j kt h -> p (j kt h)"),
                            in_=psc, func=EXP, scale=1.0 / (np.sqrt(D).item() * SCL))

                    if ph == 0:
                        # val[k, o] (k on partitions within kt): emitted after
                        # the first phase's blocks so it fills PE idle gaps
                        # between DMA-paced blocks, not the stream start
                        for kt in range(KT):
                            for vh in range(2):
                                pv = pscp.tile([P, H // 2], f32, tag="sc")
                                for ic in range(FC):
                                    nc.tensor.matmul(
                                        pv, hsT_sb[:, ic, kt * P:(kt + 1) * P],
                                        wv_sb[:, ic, vh * (H // 2):(vh + 1) * (H // 2)],
                                        start=(ic == 0), stop=(ic == FC - 1))
                                nc.vector.tensor_copy(
                                    out=val_sb[:, kt, vh * (H // 2):(vh + 1) * (H // 2)],
                                    in_=pv)

                    # den + ctx matmuls run unhindered into packed psum banks;
                    # the normalization muls use engine-disjoint out tiles
                    # (alternating engines on one tile serializes via WAW sems)
                    q0, nq = b0 * QB, (b1 - b0) * QB
                    qs = slice(q0, q0 + nq)
                    out_sb = persist.tile([nq, H], bf16, name=f"out_sb{ph}")
                    den_r = persist.tile([nq, NH], f32, name=f"denr{ph}")
                    pd = pden.tile([nq, NH], f32, tag="den")
                    for h in range(NH):
                        for kt in range(KT):
                            nc.tensor.matmul(
                                pd[:, h:h + 1], expT[:, qs, kt, h], ones_bf,
                                start=(kt == 0), stop=(kt == KT - 1))
                    nc.vector.reciprocal(out=den_r, in_=pd)

                    pc0 = pctx.tile([nq, 8 * D], f32, tag="cb0", name="pc0")
                    pc1 = pctx.tile([nq, 4 * D], f32, tag="cb1", name="pc1")
                    pcs = [pc0, pc1]
                    # small bank (pc1) first: its mul overlaps the pc0 matmuls
                    for h in list(range(8, NH)) + list(range(8)):
                        bank, off = (pcs[0], h * D) if h < 8 else (pcs[1], (h - 8) * D)
                        for kt in range(KT):
                            nc.tensor.matmul(
                                bank[:, off:off + D], expT[:, qs, kt, h],
                                val_sb[:, kt, h * D:(h + 1) * D],
                                start=(kt == 0), stop=(kt == KT - 1))
                    # normalize with two wide tensor_tensor muls; den_r is
                    # free-dim-broadcast (stride 0 over d) to [nq, h, 64];
                    # both on DVE (same engine: no cross-engine WAW on out_sb)
                    den_b = den_r.rearrange("q (h o) -> q h o", o=1)
                    nc.vector.tensor_mul(
                        out=out_sb[:, HD:].rearrange("q (h o) -> q h o", o=D),
                        in0=pc1, in1=den_b[:, 8:12, :].broadcast_to([nq, 4, D]))
                    nc.vector.tensor_mul(
                        out=out_sb[:, :HD].rearrange("q (h o) -> q h o", o=D),
                        in0=pc0, in1=den_b[:, 0:8, :].broadcast_to([nq, 8, D]))

                    nc.sync.dma_start(out=out[qs, :], in_=out_sb)

    nc.compile()
    return nc


def make_in_maps(inputs):
    import ml_dtypes
    bf = ml_dtypes.bfloat16
    f8 = ml_dtypes.float8_e3m4 if REL_FP8 else ml_dtypes.bfloat16

    hs = np.asarray(inputs["hidden_states"], np.float32)[0]          # [S, H]
    rel = np.asarray(inputs["rel_embedding"], np.float32)[0]         # [S, S, H]
    msk = np.asarray(inputs["attention_mask"], np.float32).reshape(S)

    # rel -> per-core [NQB, P, QB, FC, S] f8, relT[qb,p,j,fc,k] = rel[q, k, fc*128+p]
    rel_q = rel.astype(f8).reshape(NCORES, NQB, QB, S, FC, P)
    rel_t = np.ascontiguousarray(rel_q.transpose(0, 1, 5, 2, 4, 3))

    f8w = ml_dtypes.float8_e3m4 if W_FP8 else bf
    wscl = 64.0 if W_FP8 else 1.0

    def t_po(a, dt=bf):  # [O, I] -> [P, FC(I), O] with partition = i within chunk
        return np.ascontiguousarray(
            a.astype(dt).T.reshape(FC, P, -1).transpose(1, 0, 2))

    def t_nat(a, dt=bf):  # [O, I] -> [P, FC(O), I] natural rows on partitions
        return np.ascontiguousarray(
            a.astype(dt).reshape(FC, P, -1).transpose(1, 0, 2))

    qkr = np.ascontiguousarray(np.stack([
        t_po(np.asarray(inputs["Wq"], np.float32) * wscl, f8w),
        t_po(np.asarray(inputs["Wk"], np.float32) * wscl, f8w),
        t_nat(np.asarray(inputs["Wr"], np.float32) * wscl, f8w),
    ], axis=1))                                                       # [P,3,FC,H]
    uvm = np.ascontiguousarray(np.concatenate([
        np.asarray(inputs["u"], np.float32).reshape(FC, P).T * wscl,
        np.asarray(inputs["v"], np.float32).reshape(FC, P).T * wscl,
        (msk * 8.0 * SCL).reshape(KT, P).T,
    ], axis=1))                                                       # [P,2FC+KT]
    hsT = t_po(hs)                                                    # [P, FC, S]
    common = {"qkr": qkr, "uvm": uvm,
              "wvT": t_po(np.asarray(inputs["Wv"], np.float32))}
    in_maps = []
    for c in range(NCORES):
        sl = slice(c * SQ, (c + 1) * SQ)
        in_maps.append({
            **common,
            "hsc": np.ascontiguousarray(
                np.concatenate([hsT, t_po(hs[sl])], axis=2)),
            "relT": rel_t[c],
        })
    return in_maps


def kernel(**inputs):
    if "nc" not in _CACHED:
        _CACHED["nc"] = build_kernel()
    nc = _CACHED["nc"]
    in_maps = make_in_maps(inputs)

    from concourse.bass_utils import run_bass_kernel_spmd
    res = run_bass_kernel_spmd(nc, in_maps, list(range(NCORES)))
    parts = [np.asarray(res.results[c]["out"]).astype(np.float32)
             for c in range(NCORES)]
    return np.concatenate(parts, axis=0)[None]
